# revision 3
# baseline (speedup 1.0000x reference)
"""Trainium2 Bass kernel v2 for CrossGeometricStructureEmbedding.

Math per point n, anchor k:
  d_idx = |p_n - a_k| / 0.2,  a_idx = atan2(|u x v|, u.v) * 12/pi
  out[n] = max_k(Wd@emb(d_idx)) + max_k(Wa@emb(a_idx)) + bd + ba

v2 design (vs v1 Chebyshev/arccos):
  * The 256-dim sinusoidal embedding is compressed in a FOURIER basis fit
    directly in x (d_idx / a_idx): emb(x) ~= C @ sin(2pi(f_j x + phi_j))
    with dyadic frequencies f_j = k_j/64 (d, m=64) and k_j/32 (a, m=32);
    residual ~1e-12. No arccos/arcsin chain at all.
  * The per-(j,pair) affine t_j = f_j*x + phi_j runs on the PE as a rank-5
    matmul (moving rows: round(4x) and x-round(4x)/4 per channel + ones),
    with dyadic stationary coefficients so f32r evaluation is exact.
  * Geometry uses Lagrange |u x v|^2 = |u|^2|v|^2 - (u.v)^2; the three
    quadratics come from rank-5 PE matmuls ([64k, 512n] layout).
  * Range reduction r = (t mod 1) - 0.5 on DVE, Sin on ACT, 4 f32r
    projection matmuls, k-max reduces split DVE-direct / ACT-copy+Q7-tree.
"""
import os
import sys

sys.path.insert(0, "/opt/trn_rl_repo")

import numpy as np
import concourse.bacc as bacc
import concourse.bass as bass
import concourse.tile as tile
from concourse import mybir
from concourse.bass_utils import run_bass_kernel_spmd

F32 = mybir.dt.float32
F32R = mybir.dt.float32r
I32 = mybir.dt.int32
AF = mybir.ActivationFunctionType
OP = mybir.AluOpType

NCORES = 8
N = 4096
NC_PTS = N // NCORES          # 512 points per core
K = 64
HIDDEN = 256
SIGMA_D = 0.2
FACTOR_A = 180.0 / (15.0 * np.pi)
TWO_PI = float(2.0 * np.pi)
RBIG = float(1.5 * 2.0**23)   # round-to-nearest-int magic constant

M_D, M_A = 64, 32
MB = M_D + M_A
P_D, P_A = 64.0, 32.0         # dyadic periods (in d_idx / a_idx units)
LO_D, HI_D = -0.5, 37.0
LO_A, HI_A = -0.5, 12.5
OFF = 2.0                     # keeps t positive (mod semantics uniform)

# range reduction: u = round(t) via ACT f32->i32 cast; r = t - u on DVE.

_DIV = np.exp(np.arange(0, HIDDEN, 2) * (-np.log(10000.0) / HIDDEN))


def _emb_grid(x):
    om = x[:, None] * _DIV
    return np.stack([np.sin(om), np.cos(om)], -1).reshape(len(x), HIDDEN)


def _fit_fourier(lo, hi, m, P, grid_n=8000):
    ks, phis = [0.0], [0.25]
    q = 1
    while len(ks) < m:
        ks.append(float(q)); phis.append(0.25)
        if len(ks) < m:
            ks.append(float(q)); phis.append(0.0)
        q += 1
    ks = np.array(ks); phis = np.array(phis)
    xg = np.linspace(lo, hi, grid_n)
    B = np.sin(2 * np.pi * (np.outer(xg, ks / P) + phis))
    C, *_ = np.linalg.lstsq(B, _emb_grid(xg), rcond=None)
    return ks, phis, C  # emb(x) ~= sin(2pi(k/P x + phi)) @ C


_KS_D, _PHI_D, _C_D = _fit_fourier(LO_D, HI_D, M_D, P_D)
_KS_A, _PHI_A, _C_A = _fit_fourier(LO_A, HI_A, M_A, P_A)

_NC_CACHE = {}


def _build_nc():
    nc = bacc.Bacc("TRN2", target_bir_lowering=False, debug=False,
                   num_devices=NCORES)
    mvgeo = nc.declare_dram_parameter("mvgeo", [5, 512], F32, isOutput=False)
    geow = nc.declare_dram_parameter("geow", [5, 192], F32, isOutput=False)
    affw = nc.declare_dram_parameter("affw", [128, 96], F32R, isOutput=False)
    wlhs = nc.declare_dram_parameter("wlhs", [MB, 512], F32R, isOutput=False)
    biasd = nc.declare_dram_parameter("biasd", [128, 2], F32, isOutput=False)
    outT = nc.declare_dram_parameter("outT", [2, 128, 512], F32, isOutput=True)

    NCH = 64

    with tile.TileContext(nc) as tc:
        with (
            tc.tile_pool(name="sg", bufs=1) as sg,
            tc.tile_pool(name="geo", bufs=1) as geo,
            tc.tile_pool(name="dram", bufs=1, space="DRAM") as dr,
            tc.tile_pool(name="ptt", bufs=2, space="PSUM") as ptt,
            tc.tile_pool(name="ppd", bufs=2, space="PSUM") as ppd,
            tc.tile_pool(name="ppa", bufs=1, space="PSUM") as ppa,
            tc.tile_pool(name="mvp", bufs=2) as mvp,
            tc.tile_pool(name="rp", bufs=3) as rp,
            tc.tile_pool(name="up", bufs=3) as up,
            tc.tile_pool(name="tsp", bufs=3) as tsp,
            tc.tile_pool(name="btp", bufs=3) as btp,
        ):
            mvg_sb = sg.tile([5, 512], F32, name="mvg_sb")
            geow_sb = sg.tile([5, 192], F32, name="geow_sb")
            affw_sb = sg.tile([128, 96], F32R, name="affw_sb")
            wlhs_sb = sg.tile([MB, 512], F32R, name="wlhs_sb")
            bias_sb = sg.tile([128, 2], F32, name="bias_sb")
            mxd = sg.tile([128, 2, 512], F32, name="mxd")
            mxa = sg.tile([128, 2, 512], F32, name="mxa")
            ones4k = sg.tile([128, 256], F32, name="ones4k")
            thrd2 = dr.tile([5, 32768], F32, name="thrd2")

            nc.sync.dma_start(mvg_sb[:], mvgeo[:])
            nc.sync.dma_start(geow_sb[:], geow[:])
            nc.sync.dma_start(affw_sb[:], affw[:])
            nc.sync.dma_start(wlhs_sb[:], wlhs[:])
            nc.sync.dma_start(bias_sb[:], biasd[:])
            nc.gpsimd.memset(ones4k[:], 1.0)

            # PE warm-up: ~4us of back-to-back matmuls flips the HAM clock
            # gate to 8/8 before the real work (it then never re-idles long
            # enough to drop back).
            for w in range(10):
                wt = ptt.tile([96, 512], F32, name="tps")
                nc.tensor.matmul(wt[0:96, :], wlhs_sb[0:64, 0:96],
                                 wlhs_sb[0:64, 0:512],
                                 start=True, stop=True)

            # -------- geometry: points on partitions, [128, 4g, 64k] ------
            gA = ppd.tile([128, 1024], F32, name="psd")   # usq | vsq
            gB = ppa.tile([128, 1024], F32, name="psa")   # dot | -
            for g in range(4):
                pslice = mvg_sb[:, g * 128:(g + 1) * 128]
                nc.tensor.matmul(gA[:, g * 64:g * 64 + 64], pslice,
                                 geow_sb[:, 0:64], start=True, stop=True)
                nc.tensor.matmul(gA[:, 256 + g * 64:256 + g * 64 + 64],
                                 pslice, geow_sb[:, 64:128],
                                 start=True, stop=True)
                nc.tensor.matmul(gB[:, g * 64:g * 64 + 64], pslice,
                                 geow_sb[:, 128:192], start=True, stop=True)
            usq = gA[:, 0:256]
            vsq_ps = gA[:, 256:512]
            dot_ps = gB[:, 0:256]

            def gt(name):
                return geo.tile([128, 256], F32, name=name)

            xd = gt("xd"); h4d = gt("h4d"); xlod = gt("xlod")
            vsq = gt("vsq"); dsb = gt("dsb"); m1 = gt("m1"); d2 = gt("d2")
            i2 = gt("i2"); scr = gt("scr"); s_ = gt("s_"); asq = gt("asq")
            arg = gt("arg"); mask = gt("mask"); fsg = gt("fsg")
            args = gt("args"); atr = gt("atr"); ang = gt("ang")
            xa = gt("xa"); h4a = gt("h4a"); xloa = gt("xloa")
            tmp = gt("tmp")

            # d chain: x_d = 5*dist = sqrt(25*usq)
            nc.scalar.activation(xd[:], usq, AF.Sqrt, scale=25.0)
            nc.vector.tensor_scalar(out=tmp[:], in0=xd[:], scalar1=4.0,
                                    scalar2=RBIG, op0=OP.mult, op1=OP.add)
            nc.vector.tensor_scalar(out=h4d[:], in0=tmp[:], scalar1=RBIG,
                                    scalar2=None, op0=OP.subtract)
            nc.vector.scalar_tensor_tensor(out=xlod[:], in0=h4d[:],
                                           scalar=-0.25, in1=xd[:],
                                           op0=OP.mult, op1=OP.add)

            # a chain: |cross|^2 = usq*vsq - dot^2 (Lagrange)
            nc.scalar.copy(vsq[:], vsq_ps)
            nc.scalar.copy(dsb[:], dot_ps)
            nc.vector.tensor_tensor(out=m1[:], in0=usq, in1=vsq[:],
                                    op=OP.mult)
            nc.scalar.activation(d2[:], dsb[:], AF.Square)
            nc.vector.reciprocal_approx_accurate(i2[:], d2[:], scr[:])
            nc.vector.tensor_tensor(out=s_[:], in0=m1[:], in1=i2[:],
                                    op=OP.mult)
            nc.vector.tensor_scalar(out=asq[:], in0=s_[:], scalar1=1.0,
                                    scalar2=0.0, op0=OP.subtract, op1=OP.max)
            nc.scalar.activation(arg[:], asq[:], AF.Sqrt)
            nc.vector.tensor_scalar(out=mask[:], in0=dsb[:], scalar1=0.0,
                                    scalar2=None, op0=OP.is_lt)
            nc.vector.tensor_scalar(out=fsg[:], in0=mask[:], scalar1=-2.0,
                                    scalar2=1.0, op0=OP.mult, op1=OP.add)
            nc.vector.tensor_tensor(out=args[:], in0=arg[:], in1=fsg[:],
                                    op=OP.mult)
            nc.scalar.activation(atr[:], args[:], AF.Arctan)
            nc.vector.scalar_tensor_tensor(out=ang[:], in0=mask[:],
                                           scalar=float(np.pi), in1=atr[:],
                                           op0=OP.mult, op1=OP.add)
            nc.vector.tensor_scalar(out=xa[:], in0=ang[:],
                                    scalar1=float(FACTOR_A), scalar2=None,
                                    op0=OP.mult)
            nc.vector.tensor_scalar(out=tmp[:], in0=xa[:], scalar1=4.0,
                                    scalar2=RBIG, op0=OP.mult, op1=OP.add)
            nc.vector.tensor_scalar(out=h4a[:], in0=tmp[:], scalar1=RBIG,
                                    scalar2=None, op0=OP.subtract)
            nc.vector.scalar_tensor_tensor(out=xloa[:], in0=h4a[:],
                                           scalar=-0.25, in1=xa[:],
                                           op0=OP.mult, op1=OP.add)

            # relayout rows to DRAM: thrd2[r, n*64+k] = row[k, n]
            for r, srct in enumerate((h4d, xlod, h4a, xloa, ones4k)):
                eng = nc.sync if r % 2 == 0 else nc.gpsimd
                eng.dma_start(
                    out=thrd2[r:r + 1, :].rearrange(
                        "a (g p k) -> (a p) g k", g=4, k=64),
                    in_=srct[:].rearrange("p (g k) -> p g k", k=64))

            # ---------------- pipelined chunk loop ------------------------
            tps = {}; rt = {}; ut = {}; bt = {}; psd = {}; psa = {}
            mv = {}; cpd = {}; w1 = {}; w2 = {}; tsb = {}

            def s_mv(rr):
                t = mvp.tile([101, 4096], F32, name="mv")
                eng = nc.sync if rr % 2 == 0 else nc.gpsimd
                eng.dma_start(t[96:101, :],
                              thrd2[0:5, rr * 4096:(rr + 1) * 4096])
                mv[rr] = t

            def s_aff(c):
                rr, cc = divmod(c, 8)
                t = ptt.tile([96, 512], F32, name="tps")
                nc.tensor.matmul(t[:], affw_sb[96:101, 0:96],
                                 mv[rr][96:101, cc * 512:(cc + 1) * 512]
                                 .bitcast(F32R),
                                 start=True, stop=True,
                                 tile_position=(96, 0))
                tps[c] = t

            def s_cast(c):
                # evacuate t to SBUF (ScalarE); round on GPSIMD (round-trick)
                ts = tsp.tile([96, 512], F32, name="ts")
                nc.scalar.copy(ts[:], tps[c][:])
                tsb[c] = ts
                t = up.tile([96, 512], I32, name="ut")
                nc.scalar.copy(t[:], tps[c][:])
                ut[c] = t
                tps.pop(c)

            def s_sub(c):
                t = rp.tile([96, 512], F32, name="rt")
                nc.gpsimd.tensor_tensor(out=t[:], in0=tsb[c][:],
                                        in1=ut[c][:], op=OP.subtract)
                rt[c] = t
                tsb.pop(c)
                ut.pop(c)

            def s_sin(c):
                t = btp.tile([96, 512], F32R, name="bt")
                nc.scalar.activation(t[:], rt[c][:], AF.Sin, scale=TWO_PI)
                bt[c] = t
                rt.pop(c)

            def s_proj(c):
                d = ppd.tile([128, 1024], F32, name="psd")
                a = ppa.tile([128, 1024], F32, name="psa")
                b = bt[c]
                nc.tensor.matmul(d[:, 0:512], wlhs_sb[0:M_D, 0:128],
                                 b[0:M_D, :], start=True, stop=True)
                nc.tensor.matmul(a[:, 0:512], wlhs_sb[M_D:MB, 256:384],
                                 b[M_D:MB, :], start=True, stop=True)
                nc.tensor.matmul(d[:, 512:1024], wlhs_sb[0:M_D, 128:256],
                                 b[0:M_D, :], start=True, stop=True)
                nc.tensor.matmul(a[:, 512:1024], wlhs_sb[M_D:MB, 384:512],
                                 b[M_D:MB, :], start=True, stop=True)
                psd[c] = d
                psa[c] = a
                bt.pop(c)

            def s_red_a(c):
                nc.vector.tensor_reduce(
                    mxa[:, :, c * 8:(c + 1) * 8],
                    psa[c].rearrange("p (t n k) -> p t n k", t=2, k=K),
                    axis=mybir.AxisListType.X, op=OP.max)
                psa.pop(c)

            def s_red_d0(c):
                nc.vector.tensor_reduce(
                    mxd[:, :, c * 8:(c + 1) * 8],
                    psd[c].rearrange("p (t n k) -> p t n k", t=2, k=K),
                    axis=mybir.AxisListType.X, op=OP.max)

            def s_cp(c):
                psd.pop(c)



            def _in(c):
                return 0 <= c < NCH

            for s in range(NCH + 6):
                if s < NCH and s % 8 == 0:
                    s_mv(s // 8)
                if _in(s - 5):
                    s_proj(s - 5)
                if _in(s - 5):
                    s_red_a(s - 5)
                    s_red_d0(s - 5)
                if _in(s - 3):
                    s_sub(s - 3)
                if _in(s - 4):
                    s_sin(s - 4)
                if _in(s - 2):
                    s_cast(s - 2)
                if _in(s - 5):
                    s_cp(s - 5)
                if _in(s - 1):
                    s_aff(s - 1)

            # ---------------- finale --------------------------------------
            o0 = geo.tile([128, 512], F32, name="o0")
            o1 = geo.tile([128, 512], F32, name="o1")
            nc.vector.scalar_tensor_tensor(out=o0[:], in0=mxd[:, 0, :],
                                           scalar=bias_sb[:, 0:1],
                                           in1=mxa[:, 0, :],
                                           op0=OP.add, op1=OP.add)
            nc.vector.scalar_tensor_tensor(out=o1[:], in0=mxd[:, 1, :],
                                           scalar=bias_sb[:, 1:2],
                                           in1=mxa[:, 1, :],
                                           op0=OP.add, op1=OP.add)
            nc.sync.dma_start(out=outT[0], in_=o0[:])
            nc.sync.dma_start(out=outT[1], in_=o1[:])

    nc.compile()
    return nc


def _host_inputs(points, anchor_points, cor_score, Wa, ba, Wd, bd):
    p = np.ascontiguousarray(points[0], dtype=np.float32)        # (4096, 3)
    a = np.asarray(anchor_points[0], dtype=np.float64)           # (64, 3)
    b = np.roll(a, -1, axis=0)

    geow = np.zeros((5, 192), np.float32)
    geow[0, 0:64] = 1.0
    geow[1:4, 0:64] = -2.0 * a.T
    geow[4, 0:64] = (a * a).sum(-1)
    geow[0, 64:128] = 1.0
    geow[1:4, 64:128] = -2.0 * b.T
    geow[4, 64:128] = (b * b).sum(-1)
    geow[0, 128:192] = 1.0
    geow[1:4, 128:192] = -(a + b).T
    geow[4, 128:192] = (a * b).sum(-1)

    affw = np.zeros((128, 96), np.float32)
    affw[96, 0:M_D] = _KS_D / 256.0
    affw[97, 0:M_D] = _KS_D / 64.0
    affw[98, M_D:MB] = _KS_A / 128.0
    affw[99, M_D:MB] = _KS_A / 32.0
    affw[100, 0:M_D] = _PHI_D + OFF
    affw[100, M_D:MB] = _PHI_A + OFF

    G_d = (_C_D @ np.asarray(Wd, np.float64).T).astype(np.float32)
    G_a = (_C_A @ np.asarray(Wa, np.float64).T).astype(np.float32)
    wlhs = np.zeros((MB, 512), np.float32)
    wlhs[0:M_D, 0:128] = G_d[:, 0:128]
    wlhs[0:M_D, 128:256] = G_d[:, 128:256]
    wlhs[M_D:MB, 256:384] = G_a[:, 0:128]
    wlhs[M_D:MB, 384:512] = G_a[:, 128:256]

    bsum = (np.asarray(bd) + np.asarray(ba)).astype(np.float32)
    biasd = np.stack([bsum[0:128], bsum[128:256]], axis=1).copy()

    in_maps = []
    for core in range(NCORES):
        pc = p[core * NC_PTS:(core + 1) * NC_PTS]    # (512, 3)
        mvgeo = np.empty((5, 512), np.float32)
        mvgeo[0] = (pc.astype(np.float64) ** 2).sum(-1)
        mvgeo[1:4] = pc.T
        mvgeo[4] = 1.0
        in_maps.append({
            "mvgeo": mvgeo,
            "geow": geow,
            "affw": affw,
            "wlhs": wlhs,
            "biasd": biasd,
        })
    return in_maps


def _install_trace_hook():
    # Dev-only: register the axon NTFF profile hook so trace=True yields
    # HW exec times. Missing pieces degrade silently (trace is skipped).
    try:
        import types
        import antenv
        if "antenv.axon_hooks" in sys.modules:
            return
        store = [None]
        m = types.ModuleType("antenv.axon_hooks")
        m.set_axon_ntff_profile_hook = lambda h: store.__setitem__(0, h)
        m.get_axon_ntff_profile_hook = lambda: store[0]
        sys.modules["antenv.axon_hooks"] = m
        antenv.axon_hooks = m
        sys.path.insert(0, "/root/.axon_site")
        from trn_agent_boot.trn_boot import _ntff_profile_via_ctypes
        m.set_axon_ntff_profile_hook(
            _ntff_profile_via_ctypes("/opt/axon/libaxon_pjrt.so"))
    except Exception:
        pass


def kernel(points, anchor_points, cor_score, Wa, ba, Wd, bd, _timing=None):
    if "nc" not in _NC_CACHE:
        _NC_CACHE["nc"] = _build_nc()
    nc = _NC_CACHE["nc"]
    in_maps = _host_inputs(points, anchor_points, cor_score, Wa, ba, Wd, bd)
    if _timing is not None:
        _install_trace_hook()
    res = run_bass_kernel_spmd(nc, in_maps, core_ids=list(range(NCORES)),
                               trace=_timing is not None)
    _NC_CACHE["last_res"] = res
    if _timing is not None:
        _timing.append(res.exec_time_ns)
    out = np.empty((N, HIDDEN), np.float32)
    for core in range(NCORES):
        ot = res.results[core]["outT"]          # (2, 128, 512)
        blk = out[core * NC_PTS:(core + 1) * NC_PTS]
        blk[:, 0:128] = ot[0].T
        blk[:, 128:256] = ot[1].T
    return out.reshape(1, N, HIDDEN)
